# revision 7
# baseline (speedup 1.0000x reference)
"""AttentionGNN Trainium2 kernel: 16 graphs data-parallel over 8 NeuronCores.

Math (per side, per layer):
  msg[e,:] = sum_k ehp[e,k] * (hh[src[e],:] @ Wcat[k])     (Wcat[16]=efb, ehp[:,16]=1)
  agg = segsum(msg, dst)/deg + convb ; h = LN(lrelu(agg)@outW+outb)*g+b
On device, per 128-edge chunk:
  G = hh_src_chunk @ WtCat   (WtCat[f, o*17+k])  -> PSUM [128, 1088]
  prod = G * ehp_broadcast   (DVE, strided AP)   -> SBUF bf16
  msg  = reduce_k(prod)                          -> [128, 64] f32
  aggT[o, d] += msg^T @ onehot(dst)              (PE accumulation in PSUM)
"""

import sys, os
sys.path.insert(0, "/opt/trn_rl_repo")

import numpy as np
import ml_dtypes
from contextlib import ExitStack

import concourse.bass as bass
import concourse.bacc as bacc
import concourse.tile as tile
from concourse import mybir
from concourse.masks import make_identity

F = 64
NL = 3
B = 16
NCORES = 8
GPC = 2  # graphs per core
EPS = 1e-5

SIDES = {
    "lig": dict(npg=64, epg=512, nin=32),
    "rec": dict(npg=256, epg=2048, nin=40),
}
for s in SIDES.values():
    s["N"] = s["npg"] * GPC
    s["E"] = s["epg"] * GPC
    s["crow"] = min(128, s["npg"])
    s["nch"] = s["N"] // s["crow"]
    s["ech"] = s["E"] // 128

BF = mybir.dt.bfloat16
F32 = mybir.dt.float32
I32 = mybir.dt.int32


def _bcast(ap, p):
    """slice first p partitions of a replicated [128, X] tile."""
    return ap[:p, :]


def build_module():
    nc = bacc.Bacc("TRN2", target_bir_lowering=False, debug=False, enable_asserts=False)

    dram = {}

    def din(name, shape, dt):
        dram[name] = nc.dram_tensor(name, shape, dt, kind="ExternalInput").ap()
        return dram[name]

    for sd, cfg in SIDES.items():
        N, E, nin = cfg["N"], cfg["E"], cfg["nin"]
        din(f"{sd}_x", [N, nin], F32)
        din(f"{sd}_e", [E, 16], F32)
        din(f"{sd}_src", [128, cfg["ech"]], I32)
        din(f"{sd}_dst", [128, cfg["ech"]], I32)
        din(f"{sd}_emb_nW", [nin, F], BF)
        din(f"{sd}_emb_nb", [1, F], F32)
        din(f"{sd}_emb_eW", [16, 16], BF)
        din(f"{sd}_emb_eb", [1, 16], F32)
        din(f"{sd}_hidW", [F, F], BF)
        din(f"{sd}_hidb", [1, F], F32)
        din(f"{sd}_ln0g", [1, F], F32)
        din(f"{sd}_ln0b", [1, F], F32)
        din(f"{sd}_WtCat", [NL, F, 17 * F], BF)
        din(f"{sd}_convb", [NL, F], F32)  # used as [64,1] column
        din(f"{sd}_outW", [NL, F, F], BF)
        din(f"{sd}_outb", [NL, 1, F], F32)
        din(f"{sd}_lng", [NL, 1, F], F32)
        din(f"{sd}_lnb", [NL, 1, F], F32)

    op_out = nc.dram_tensor("op_out", [GPC, 64, 256], F32, kind="ExternalOutput").ap()
    outv = nc.dram_tensor("outv", [GPC, 1], F32, kind="ExternalOutput").ap()

    # internal scratch for gather source (lrelu(h)) per side per layer
    hhd = {
        (sd, i): nc.dram_tensor(f"hhd_{sd}_{i}", [SIDES[sd]["N"], F], F32, kind="Internal").ap()
        for sd in SIDES for i in range(NL)
    }

    with tile.TileContext(nc) as tc, ExitStack() as ctx:
        single = ctx.enter_context(tc.tile_pool(name="single", bufs=1))
        work = ctx.enter_context(tc.tile_pool(name="work", bufs=3))
        big = ctx.enter_context(tc.tile_pool(name="big", bufs=1))
        psT = ctx.enter_context(tc.tile_pool(name="psT", bufs=2, space="PSUM"))
        psG = ctx.enter_context(tc.tile_pool(name="psG", bufs=1, space="PSUM"))
        psA = ctx.enter_context(tc.tile_pool(name="psA", bufs=1, space="PSUM"))

        ident = single.tile([128, 128], F32)
        make_identity(nc, ident[:])
        ones_col_bf = single.tile([128, 1], BF)
        nc.vector.memset(ones_col_bf[:], 1.0)
        ones_col_f = single.tile([128, 1], F32)
        nc.vector.memset(ones_col_f[:], 1.0)
        eps_col = single.tile([128, 1], F32)
        nc.vector.memset(eps_col[:], EPS)
        iota_i = single.tile([128, 256], I32)
        nc.gpsimd.iota(iota_i[:], pattern=[[1, 256]], base=0, channel_multiplier=0)
        iota_f = single.tile([128, 256], F32)
        nc.vector.tensor_copy(iota_f[:], iota_i[:])

        def transpose_to(pool, src_ap, out_dt, rows, cols=128):
            """src [cols, rows] -> SBUF [rows, cols] in out_dt via PE."""
            ps = psT.tile([128, 128], F32, tag="ps")
            nc.tensor.transpose(out=ps[:rows, :cols], in_=src_ap, identity=ident[:cols, :cols])
            sb = pool.tile([rows, 128], out_dt, tag=f"tr{rows}{out_dt}")
            nc.vector.tensor_copy(sb[:rows, :cols], ps[:rows, :cols])
            return sb

        S = {}  # per-side persistent state
        for sd, cfg in SIDES.items():
            N, E, nch, ech, npg, nin = cfg["N"], cfg["E"], cfg["nch"], cfg["ech"], cfg["npg"], cfg["nin"]
            crow = cfg["crow"]
            st = {}
            S[sd] = st
            # ---- load weights to SBUF ----
            for nm, shape, dt in [
                ("emb_nW", [nin, F], BF), ("emb_eW", [16, 16], BF), ("hidW", [F, F], BF),
            ]:
                t = single.tile(shape, dt, tag=f"{sd}{nm}")
                nc.sync.dma_start(out=t[:], in_=dram[f"{sd}_{nm}"][:])
                st[nm] = t
            for nm, width in [("emb_nb", F), ("emb_eb", 16), ("hidb", F),
                              ("ln0g", F), ("ln0b", F)]:
                t = single.tile([128, width], F32, tag=f"{sd}{nm}")
                rap = dram[f"{sd}_{nm}"][:]
                nc.gpsimd.dma_start(out=t[:], in_=bass.AP(rap.tensor, rap.offset, [[0, 128]] + rap.ap[1:]))
                st[nm] = t
            st["WtCat"] = []
            st["outW"] = []
            st["rows"] = {}
            for i in range(NL):
                w = single.tile([F, 17 * F], BF, tag=f"{sd}WtC{i}")
                nc.sync.dma_start(out=w[:], in_=dram[f"{sd}_WtCat"][i])
                st["WtCat"].append(w)
                ow = single.tile([F, F], BF, tag=f"{sd}oW{i}")
                nc.sync.dma_start(out=ow[:], in_=dram[f"{sd}_outW"][i])
                st["outW"].append(ow)
                cb = single.tile([F, 1], F32, tag=f"{sd}cb{i}")
                nc.sync.dma_start(out=cb[:], in_=dram[f"{sd}_convb"][i, :, None])
                st.setdefault("convb", []).append(cb)
                for nm in ("outb", "lng", "lnb"):
                    t = single.tile([128, F], F32, tag=f"{sd}{nm}{i}")
                    rap = dram[f"{sd}_{nm}"][i]
                    nc.gpsimd.dma_start(out=t[:], in_=bass.AP(rap.tensor, rap.offset, [[0, 128]] + rap.ap[1:]))
                    st["rows"][(nm, i)] = t

            # ---- indices ----
            src_sb = single.tile([128, ech], I32, tag=f"{sd}src")
            nc.sync.dma_start(out=src_sb[:], in_=dram[f"{sd}_src"][:])
            st["src"] = src_sb
            dst_sb = single.tile([128, ech], I32, tag=f"{sd}dst")
            nc.sync.dma_start(out=dst_sb[:], in_=dram[f"{sd}_dst"][:])
            dst_f = single.tile([128, ech], F32, tag=f"{sd}dstf")
            nc.vector.tensor_copy(dst_f[:], dst_sb[:])

            # ---- one-hot DT per chunk + deg ----
            DT = single.tile([128, ech, npg], BF, tag=f"{sd}DT")
            st["DT"] = DT
            for ec in range(ech):
                nc.vector.tensor_tensor(
                    out=DT[:, ec, :],
                    in0=dst_f[:, ec:ec + 1].to_broadcast([128, npg]),
                    in1=iota_f[:, :npg],
                    op=mybir.AluOpType.is_equal,
                )
            cpg = cfg["epg"] // 128  # chunks per graph
            deg_ps = psA.tile([1, GPC * npg], F32, tag=f"agg{sd}")
            for g in range(GPC):
                for j in range(cpg):
                    ec = g * cpg + j
                    nc.tensor.matmul(
                        out=deg_ps[:, g * npg:(g + 1) * npg], lhsT=ones_col_bf[:],
                        rhs=DT[:, ec, :], start=(j == 0), stop=(j == cpg - 1),
                    )
            deg_sb = single.tile([1, GPC * npg], F32, tag=f"{sd}deg")
            nc.vector.tensor_scalar_max(out=deg_sb[:], in0=deg_ps[:], scalar1=1.0)
            invdeg_r = single.tile([1, GPC * npg], F32, tag=f"{sd}ideg")
            nc.vector.reciprocal(invdeg_r[:], deg_sb[:])
            ones_row = single.tile([1, F], BF, tag=f"{sd}onesr")
            nc.vector.memset(ones_row[:], 1.0)
            invdeg_rb = single.tile([1, GPC * npg], BF, tag=f"{sd}idegb")
            nc.vector.tensor_copy(invdeg_rb[:], invdeg_r[:])
            idg_ps = psA.tile([F, GPC * npg], F32, tag=f"agg{sd}")
            nc.tensor.matmul(out=idg_ps[:], lhsT=ones_row[:], rhs=invdeg_rb[:], start=True, stop=True)
            invdeg = single.tile([F, GPC * npg], F32, tag=f"{sd}idegbc")
            nc.vector.tensor_copy(invdeg[:], idg_ps[:])
            st["invdeg"] = invdeg

            # ---- edge features ehp [128, ec, 17], col16 = 1 ----
            ehp = single.tile([128, ech, 17], F32, tag=f"{sd}ehp")
            nc.vector.memset(ehp[:], 1.0)
            st["ehp"] = ehp
            for ec in range(ech):
                ex = work.tile([128, 16], F32, tag="ex")
                nc.sync.dma_start(out=ex[:], in_=dram[f"{sd}_e"][ec * 128:(ec + 1) * 128, :])
                exT = transpose_to(work, ex[:, :16], BF, 16)
                ep = psT.tile([128, 128], F32, tag="ps")
                nc.tensor.matmul(out=ep[:128, :16], lhsT=exT[:], rhs=st["emb_eW"][:], start=True, stop=True)
                nc.vector.tensor_add(
                    out=ehp[:, ec, 0:16], in0=ep[:128, :16],
                    in1=st["emb_eb"][:, :],
                )

            # ---- node embedding -> h [nch][128, F] f32 ----
            st["h"] = []
            st["h0"] = []
            for c in range(nch):
                x = work.tile([128, nin], F32, tag="x")
                nc.sync.dma_start(out=x[:crow, :], in_=dram[f"{sd}_x"][c * crow:(c + 1) * crow, :])
                xT = transpose_to(work, x[:crow, :nin], BF, nin, crow)
                pre_ps = psG.tile([128, F], F32, tag="preps")
                nc.tensor.matmul(out=pre_ps[:crow, :], lhsT=xT[:nin, :crow], rhs=st["emb_nW"][:], start=True, stop=True)
                pre = work.tile([128, F], F32, tag="pre")
                nc.vector.tensor_add(out=pre[:crow, :], in0=pre_ps[:crow, :], in1=st["emb_nb"][:crow, :])
                tmp = work.tile([128, F], F32, tag="lrtmp")
                nc.vector.tensor_scalar_mul(out=tmp[:crow, :], in0=pre[:crow, :], scalar1=0.01)
                nc.vector.tensor_tensor(out=pre[:crow, :], in0=pre[:crow, :], in1=tmp[:crow, :], op=mybir.AluOpType.max)
                preT = transpose_to(work, pre[:crow, :F], BF, F, crow)
                h_ps = psG.tile([128, F], F32, tag="preps")
                nc.tensor.matmul(out=h_ps[:crow, :], lhsT=preT[:F, :crow], rhs=st["hidW"][:], start=True, stop=True)
                h = single.tile([128, F], F32, tag=f"{sd}h{c}")
                nc.vector.tensor_add(out=h[:crow, :], in0=h_ps[:crow, :], in1=st["hidb"][:crow, :])
                layernorm(nc, work, h, st["ln0g"], st["ln0b"], eps_col, crow)
                st["h"].append(h)
                h0t = single.tile([128, F], F32, tag=f"{sd}h0{c}")
                st["h0"].append(h0t)

        # ---- message-passing layers ----
        for i in range(NL):
            for sd, cfg in SIDES.items():
                st = S[sd]
                N, E, nch, ech, npg = cfg["N"], cfg["E"], cfg["nch"], cfg["ech"], cfg["npg"]
                crow = cfg["crow"]
                cpg = cfg["epg"] // 128
                # hh = lrelu(h) -> SBUF + DRAM
                for c in range(nch):
                    hh = work.tile([128, F], F32, tag="hh")
                    tmp = work.tile([128, F], F32, tag="lrtmp")
                    nc.vector.tensor_scalar_mul(out=tmp[:crow, :], in0=st["h"][c][:crow, :], scalar1=0.01)
                    nc.vector.tensor_tensor(out=hh[:crow, :], in0=st["h"][c][:crow, :], in1=tmp[:crow, :], op=mybir.AluOpType.max)
                    nc.sync.dma_start(out=hhd[(sd, i)][c * crow:(c + 1) * crow, :], in_=hh[:crow, :])

                # per-edge chunks: gather -> G -> msg -> segsum
                agg_all = psA.tile([F, GPC * npg], F32, tag=f"agg{sd}")
                for ec in range(ech):
                    g = ec // cpg
                    hs = work.tile([128, F], F32, tag="hs")
                    nc.gpsimd.indirect_dma_start(
                        out=hs[:], out_offset=None,
                        in_=hhd[(sd, i)][:],
                        in_offset=bass.IndirectOffsetOnAxis(ap=st["src"][:, ec:ec + 1], axis=0),
                    )
                    hsT = transpose_to(work, hs[:, :F], BF, F)
                    G_ps = psG.tile([128, 17 * F], F32, tag="G")
                    for j, (a, b) in enumerate([(0, 512), (512, 1024), (1024, 1088)]):
                        nc.tensor.matmul(out=G_ps[:, a:b], lhsT=hsT[:], rhs=st["WtCat"][i][:, a:b],
                                         start=True, stop=True)
                    prod = work.tile([128, F, 17], BF, tag="prod")
                    eh_ap = st["ehp"][:, ec, :]
                    eh_b = bass.AP(eh_ap.tensor, eh_ap.offset, [eh_ap.ap[0], [0, F], eh_ap.ap[1]])
                    nc.vector.tensor_tensor(
                        out=prod[:], in0=G_ps[:].rearrange("p (o k) -> p o k", k=17),
                        in1=eh_b, op=mybir.AluOpType.mult,
                    )
                    msg = work.tile([128, F], F32, tag="msg")
                    nc.vector.tensor_reduce(out=msg[:], in_=prod[:], axis=mybir.AxisListType.X,
                                            op=mybir.AluOpType.add)
                    msg_bf = work.tile([128, F], BF, tag="msgbf")
                    nc.vector.tensor_copy(out=msg_bf[:], in_=msg[:])
                    j = ec % cpg
                    nc.tensor.matmul(out=agg_all[:, g * npg:(g + 1) * npg], lhsT=msg_bf[:],
                                     rhs=st["DT"][:, ec, :], start=(j == 0), stop=(j == cpg - 1))

                # agg -> h update, per graph
                for g in range(GPC):
                    aggT = work.tile([F, npg], F32, tag=f"aggs{sd}")
                    nc.vector.tensor_tensor(
                        out=aggT[:], in0=agg_all[:, g * npg:(g + 1) * npg],
                        in1=st["invdeg"][:, g * npg:(g + 1) * npg],
                        op=mybir.AluOpType.mult,
                    )
                    nc.vector.tensor_scalar_add(out=aggT[:], in0=aggT[:], scalar1=st["convb"][i][:, 0:1])
                    tmp = work.tile([F, npg], F32, tag=f"aggt{sd}")
                    nc.vector.tensor_scalar_mul(out=tmp[:], in0=aggT[:], scalar1=0.01)
                    nc.vector.tensor_tensor(out=aggT[:], in0=aggT[:], in1=tmp[:], op=mybir.AluOpType.max)
                    aggT_bf = work.tile([F, npg], BF, tag=f"aggb{sd}")
                    nc.vector.tensor_copy(out=aggT_bf[:], in_=aggT[:])
                    for dc in range(npg // crow):
                        c = g * (npg // crow) + dc  # node-chunk index
                        y_ps = psG.tile([128, F], F32, tag="preps")
                        nc.tensor.matmul(out=y_ps[:crow, :], lhsT=aggT_bf[:, dc * crow:(dc + 1) * crow],
                                         rhs=st["outW"][i][:], start=True, stop=True)
                        ln_in = work.tile([128, F], F32, tag="lnin")
                        nc.vector.tensor_add(out=ln_in[:crow, :], in0=y_ps[:crow, :],
                                             in1=st["rows"][("outb", i)][:crow, :])
                        finish_layer(nc, work, ln_in, crow, st, i, eps_col,
                                     st["h"][c][:crow, :], st["h0"][c][:crow, :], i == NL - 1)

        # ---- scores ----
        ligT = big.tile([F, 128], F32, tag="ligT")
        for g in range(GPC):
            ps = psT.tile([128, 128], F32, tag="ps")
            nc.tensor.transpose(out=ps[:F, :64], in_=S["lig"]["h"][g][:64, :F], identity=ident[:64, :64])
            nc.vector.tensor_copy(out=ligT[:, g * 64:(g + 1) * 64], in_=ps[:F, :64])
        recT = big.tile([F, 512], F32, tag="recT")
        for c in range(4):
            ps = psT.tile([128, 128], F32, tag="ps")
            nc.tensor.transpose(out=ps[:F, :128], in_=S["rec"]["h"][c][:, :F], identity=ident[:])
            nc.vector.tensor_copy(out=recT[:, c * 128:(c + 1) * 128], in_=ps[:F, :128])
        for g in range(GPC):
            op_ps = psA.tile([F, 256], F32, tag="aggrec")
            nc.tensor.matmul(out=op_ps[:], lhsT=ligT[:, g * 64:(g + 1) * 64],
                             rhs=recT[:, g * 256:(g + 1) * 256], start=True, stop=True)
            op_sb = work.tile([F, 256], F32, tag="opsb")
            nc.vector.tensor_copy(out=op_sb[:], in_=op_ps[:])
            nc.sync.dma_start(out=op_out[g], in_=op_sb[:])
            rs = work.tile([F, 1], F32, tag="rs")
            nc.vector.tensor_reduce(out=rs[:], in_=op_sb[:], axis=mybir.AxisListType.X,
                                    op=mybir.AluOpType.add)
            rs_bf = work.tile([F, 1], F32, tag="rsb")
            nc.vector.tensor_copy(out=rs_bf[:], in_=rs[:])
            tot_ps = psT.tile([128, 128], F32, tag="ps")
            nc.tensor.matmul(out=tot_ps[:1, :1], lhsT=ones_col_f[:F, :], rhs=rs_bf[:], start=True, stop=True)
            tot = work.tile([1, 1], F32, tag="tot")
            nc.scalar.mul(out=tot[:], in_=tot_ps[:1, :1], mul=1.0 / (64.0 * 256.0))
            nc.sync.dma_start(out=outv[g:g + 1, :], in_=tot[:])

    nc.compile()
    return nc


def layernorm(nc, pool, h, g_row, b_row, eps_col, rows=128):
    stats = pool.tile([128, 6], F32, tag="lnstats")
    nc.vector.bn_stats(out=stats[:rows], in_=h[:rows, :])
    mv = pool.tile([128, 2], F32, tag="lnmv")
    nc.vector.bn_aggr(out=mv[:rows], in_=stats[:rows])
    sd = pool.tile([128, 1], F32, tag="lnsd")
    nc.scalar.activation(out=sd[:rows], in_=mv[:rows, 1:2], func=mybir.ActivationFunctionType.Sqrt,
                         bias=eps_col[:rows], scale=1.0)
    nc.vector.reciprocal(sd[:rows], sd[:rows])
    nc.vector.tensor_scalar(out=h[:rows, :], in0=h[:rows, :], scalar1=mv[:rows, 0:1], scalar2=sd[:rows],
                            op0=mybir.AluOpType.subtract, op1=mybir.AluOpType.mult)
    nc.vector.tensor_tensor(out=h[:rows, :], in0=h[:rows, :], in1=g_row[:rows, :],
                            op=mybir.AluOpType.mult)
    nc.vector.tensor_add(out=h[:rows, :], in0=h[:rows, :], in1=b_row[:rows, :])


def finish_layer(nc, pool, ln_in, rows, st, i, eps_col, h_dst, h0_tile, is_last):
    """LN(ln_in)*lng+lnb (+h0 if last layer); writes into h_dst; saves h0 at i==0."""
    layernorm(nc, pool, ln_in, st["rows"][("lng", i)], st["rows"][("lnb", i)], eps_col, rows)
    nc.vector.tensor_copy(out=h_dst, in_=ln_in[:rows, :])
    if is_last:
        nc.vector.tensor_add(out=h_dst, in0=h_dst, in1=h0_tile)
    if i == 0:
        nc.vector.tensor_copy(out=h0_tile, in_=ln_in[:rows, :])


# ---------------- host side ----------------

def _prep_weights(inputs, sd):
    out = {}
    W = inputs[f"{sd}_efW"].reshape(NL, 16, F, F)
    Wb = inputs[f"{sd}_efb"].reshape(NL, 1, F, F)
    Wcat = np.concatenate([W, Wb], axis=1)            # [NL, 17, f, o]
    out[f"{sd}_WtCat"] = np.ascontiguousarray(
        Wcat.transpose(0, 2, 3, 1).reshape(NL, F, F * 17)).astype(ml_dtypes.bfloat16)
    out[f"{sd}_emb_nW"] = inputs[f"{sd}_emb_nW"].astype(ml_dtypes.bfloat16)
    out[f"{sd}_emb_eW"] = inputs[f"{sd}_emb_eW"].astype(ml_dtypes.bfloat16)
    out[f"{sd}_hidW"] = inputs[f"{sd}_hidW"].astype(ml_dtypes.bfloat16)
    out[f"{sd}_outW"] = inputs[f"{sd}_outW"].astype(ml_dtypes.bfloat16)
    out[f"{sd}_emb_nb"] = inputs[f"{sd}_emb_nb"].reshape(1, F).astype(np.float32)
    out[f"{sd}_emb_eb"] = inputs[f"{sd}_emb_eb"].reshape(1, 16).astype(np.float32)
    out[f"{sd}_hidb"] = inputs[f"{sd}_hidb"].reshape(1, F).astype(np.float32)
    out[f"{sd}_ln0g"] = inputs[f"{sd}_ln0_g"].reshape(1, F).astype(np.float32)
    out[f"{sd}_ln0b"] = inputs[f"{sd}_ln0_b"].reshape(1, F).astype(np.float32)
    out[f"{sd}_convb"] = inputs[f"{sd}_convb"].astype(np.float32)
    out[f"{sd}_outb"] = inputs[f"{sd}_outb"].reshape(NL, 1, F)[:, :, :].astype(np.float32)
    out[f"{sd}_lng"] = inputs[f"{sd}_lng"].reshape(NL, 1, F).astype(np.float32)
    out[f"{sd}_lnb"] = inputs[f"{sd}_lnb"].reshape(NL, 1, F).astype(np.float32)
    return out


def make_in_maps(inputs):
    maps = []
    wshared = {}
    for sd in SIDES:
        wshared.update(_prep_weights(inputs, sd))
    for c in range(NCORES):
        m = dict(wshared)
        for sd, cfg in SIDES.items():
            N, E, npg, epg, ech = cfg["N"], cfg["E"], cfg["npg"], cfg["epg"], cfg["ech"]
            m[f"{sd}_x"] = np.ascontiguousarray(inputs[f"{sd}_x"][c * N:(c + 1) * N]).astype(np.float32)
            m[f"{sd}_e"] = np.ascontiguousarray(inputs[f"{sd}_e"][c * E:(c + 1) * E]).astype(np.float32)
            src = inputs[f"{sd}_src"][c * E:(c + 1) * E].astype(np.int64) - c * N
            dst = inputs[f"{sd}_dst"][c * E:(c + 1) * E].astype(np.int64)
            dstl = np.empty_like(dst)
            for g in range(GPC):
                dstl[g * epg:(g + 1) * epg] = dst[g * epg:(g + 1) * epg] - (c * GPC + g) * npg
            m[f"{sd}_src"] = np.ascontiguousarray(src.reshape(ech, 128).T).astype(np.int32)
            m[f"{sd}_dst"] = np.ascontiguousarray(dstl.reshape(ech, 128).T).astype(np.int32)
        maps.append(m)
    return maps


_NC_CACHE = {}


def kernel(**inputs):
    from concourse import bass_utils
    inputs = {k: np.asarray(v) for k, v in inputs.items()}
    if "nc" not in _NC_CACHE:
        _NC_CACHE["nc"] = build_module()
    nc = _NC_CACHE["nc"]
    in_maps = make_in_maps(inputs)
    res = bass_utils.run_bass_kernel_spmd(nc, in_maps, core_ids=list(range(NCORES)))
    op = np.concatenate([r["op_out"] for r in res.results], axis=0).astype(np.float32)
    outm = np.concatenate([r["outv"].reshape(-1) for r in res.results], axis=0).astype(np.float32)
    return outm, op
